# revision 3
# baseline (speedup 1.0000x reference)
"""MoE + LoRA expert FFN kernel for 8 Trainium2 NeuronCores.

Strategy (expert-parallel, host dispatch/combine):
  - E=8 experts, one expert per core. The host groups tokens by expert
    (a token appears once per distinct selected expert; duplicate
    selections collapse with summed routing weight), pads each group to
    a uniform capacity C, and ships per-core inputs:
        xT  [H, C]   tokens routed to this core's expert, transposed
        wg  [H, I]   gate_proj + 2*gate_A@gate_B   (LoRA folded)
        wu  [H, I]   up_proj   + 2*up_A@up_B
        wd  [I, H]   down_proj + 2*down_A@down_B
    and receives yT [H, C] fp32 = (silu(x@wg) * (x@wu)) @ wd, transposed.
  - Everything on device stays feature-major (features on partitions,
    tokens on the moving free dim) so no transposes are needed.
  - All matmul operands are bf16 (measured end-to-end rel err 4.3e-3 vs
    the 2e-2 gate); PSUM accumulation is fp32. bf16 halves HBM/SBUF
    traffic vs fp32r at the same PE rate.
  - The host scales each token's expert output by its routing weight and
    scatters back into the [T, H] result.

LoRA folding is exact algebra: x@W + s*(x@A)@B == x@(W + s*A@B).

Schedule per core:
  - x tiles ride the two HWDGE rings (sync/scalar), interleaved with the
    first weight group so the phase-B k-accumulation can start as soon
    as the k=0 pair lands (~9.5us) instead of waiting for a full 2MB
    SWDGE x load.
  - A short burst of dummy matmuls warms the PE clock (HAM un-throttle)
    while the first DMAs are in flight.
  - wd streams on gpsimd/SWDGE during phase B (which no longer needs
    gpsimd for x), so phase D runs with all weights resident and its
    final copy+store tail is one small token tile.
"""

import numpy as np
import ml_dtypes

E, H, I, R, TOPK = 8, 1024, 2816, 8, 2
SCALING = 2.0
NCORES = 8
KP = 128          # partition / contraction tile
NTOK = 512        # moving-dim (token) tile
BF16 = ml_dtypes.bfloat16
NDUMMY = 8        # PE-warmup matmuls before the first real one

_cache = {}


def _setup_paths():
    import sys
    for p in ("/opt/trn_rl_repo", "/root/.axon_site"):
        if p not in sys.path:
            sys.path.insert(0, p)


def _split_multi_waits(nc):
    """The walrus in this container accepts at most 1 sem wait per
    instruction (2 on EventSemaphore); Tile emits more. Rewrite each block,
    moving excess waits onto preceding single-wait NoOps on the same
    engine (engines execute in order, so semantics are preserved)."""
    _setup_paths()
    from bass_rust import SyncInfo
    from concourse import mybir

    ctr = [0]
    for f in nc.m.functions:
        for bb in f.blocks:
            insts = bb.instructions
            new = []
            changed = False
            for inst in insts:
                si = inst.sync_info
                waits = list(si.on_wait or []) if si is not None else []
                cap = 2 if isinstance(inst, mybir.InstEventSemaphore) else 1
                if len(waits) > cap:
                    changed = True
                    for w in waits[:-cap]:
                        nop = mybir.InstNoOp(
                            name=f"SW-{ctr[0]}", ins=[], outs=[])
                        ctr[0] += 1
                        nop.engine = inst.engine
                        nop.sync_info = SyncInfo(on_wait=[w], on_update=[])
                        new.append(nop)
                    inst.sync_info = SyncInfo(
                        on_wait=waits[-cap:],
                        on_update=list(si.on_update or []))
                new.append(inst)
            if changed:
                bb.instructions = new


def _token_tiles(C):
    tiles = []
    t0 = 0
    while t0 < C:
        tw = min(NTOK, C - t0)
        tiles.append((t0, tw))
        t0 += tw
    return tiles


def _build(C):
    """Build the per-core Bass program for token capacity C."""
    _setup_paths()
    import concourse.bass as bass
    import concourse.tile as tile
    from concourse import mybir

    f32 = mybir.dt.float32
    sdt = mybir.dt.bfloat16

    KH = H // KP            # 8 contraction chunks over H
    KI = I // KP            # 22 chunks over I
    HH = H // KP            # 8 output row blocks

    nc = bass.Bass("TRN2", target_bir_lowering=False, debug=False,
                   num_devices=NCORES)
    xT = nc.declare_dram_parameter("xT", [H, C], sdt, isOutput=False)
    wg = nc.declare_dram_parameter("wg", [H, I], sdt, isOutput=False)
    wu = nc.declare_dram_parameter("wu", [H, I], sdt, isOutput=False)
    wd = nc.declare_dram_parameter("wd", [I, H], sdt, isOutput=False)
    yT = nc.declare_dram_parameter("yT", [H, C], f32, isOutput=True)

    ttiles = _token_tiles(C)

    # ramped weight column groups (in i-tiles): small first for fast start
    groups = [2, 2, 2]
    while sum(groups) < KI:
        groups.append(min(4, KI - sum(groups)))
    gstart = [sum(groups[:j]) for j in range(len(groups))]
    i2q = {}
    for qq, (g0, gn) in enumerate(zip(gstart, groups)):
        for i in range(g0, g0 + gn):
            i2q[i] = (qq, i - g0)

    with tile.TileContext(nc) as tc:
        with tc.tile_pool(name="hh", bufs=1) as hp, \
             tc.tile_pool(name="wdp", bufs=1) as wdp:
            h_t = [hp.tile([KP, C], sdt, tag=f"h{i}", name=f"h{i}")
                   for i in range(KI)]
            wd_t = {}

            # ---- phase B: h = silu(x@wg) * (x@wu), feature-major [I, C]
            with tc.tile_pool(name="xp", bufs=1) as xp, \
                 tc.tile_pool(name="wst", bufs=32) as wst, \
                 tc.tile_pool(name="psB", bufs=4, space="PSUM") as psB, \
                 tc.tile_pool(name="actB", bufs=4) as actB:
                # PE warm-up: dummy matmuls un-throttle the HAM while the
                # first DMAs are in flight (must be emitted before any
                # dma-dependent matmul on the PE queue)
                wsrc = actB.tile([KP, 256], mybir.dt.bfloat16,
                                 tag="wsrc", name="wsrc")
                nc.vector.memset(wsrc, 0.0)
                wdst = psB.tile([KP, 256], f32, tag="g", name="wdst")
                for w in range(NDUMMY):
                    nc.tensor.matmul(wdst, wsrc[:, :128], wsrc,
                                     start=(w == 0), stop=(w == NDUMMY - 1))

                # x tiles + group-0 wg interleaved on the two HWDGE rings
                # in k-consumption order, so the first PSUM group's
                # k-accumulation is paced by pair arrivals, not by the
                # whole x load.
                wg_t, wu_t = {}, {}
                NG = len(groups)

                def load_w_one(mat, store, k, q, eng):
                    c0 = gstart[q] * KP
                    cw = groups[q] * KP
                    t = wst.tile([KP, 4 * KP], sdt, tag="w",
                                 name=f"w{'gu'[mat]}{k}_{q}")
                    eng.dma_start(
                        out=t[:, :cw],
                        in_=(wg if mat == 0 else wu)[
                            k * KP:(k + 1) * KP, c0:c0 + cw])
                    store[(k, q)] = t

                x_t = []
                for k in range(KH):
                    t = xp.tile([KP, C], sdt, tag=f"x{k}", name=f"x{k}")
                    eng = nc.sync if (k % 2 == 0) else nc.scalar
                    eng.dma_start(out=t, in_=xT[k * KP:(k + 1) * KP, :])
                    x_t.append(t)
                    load_w_one(0, wg_t, k, 0,
                               nc.sync if (k % 2 == 0) else nc.scalar)

                def load_w_group(q, mats=(0, 1)):
                    # allocation order must track consumption order — the
                    # shared-tag slot pool recycles FIFO
                    for k in range(KH):
                        if 0 in mats:
                            load_w_one(0, wg_t, k, q, nc.sync)
                        if 1 in mats:
                            load_w_one(1, wu_t, k, q, nc.scalar)

                load_w_group(0, mats=(1,))   # wu group 0
                load_w_group(1)
                # wd streaming on gpsimd/SWDGE: 3 tiles up-front, the rest
                # staggered through the i-loop so all of wd is resident
                # well before phase D starts.
                for i in range(3):
                    t = wdp.tile([KP, H], sdt, tag=f"wds{i}",
                                 name=f"wds{i}")
                    nc.gpsimd.dma_start(
                        out=t, in_=wd[i * KP:(i + 1) * KP, :])
                    wd_t[i] = t

                for i in range(KI):
                    q, r = i2q[i]
                    if r == 0 and q + 2 < NG:
                        load_w_group(q + 2)
                    # stagger one wd tile per i-tile
                    wdi = i + 3
                    if wdi < KI:
                        t = wdp.tile([KP, H], sdt, tag=f"wds{wdi}",
                                     name=f"wds{wdi}")
                        nc.gpsimd.dma_start(
                            out=t, in_=wd[wdi * KP:(wdi + 1) * KP, :])
                        wd_t[wdi] = t
                    isl = slice(r * KP, (r + 1) * KP)
                    for ti, (t0, tw) in enumerate(ttiles):
                        g_ps = psB.tile([KP, tw], f32, tag="g",
                                        name=f"g{i}_{t0}")
                        u_ps = psB.tile([KP, tw], f32, tag="u",
                                        name=f"u{i}_{t0}")
                        for k in range(KH):
                            nc.tensor.matmul(
                                g_ps, wg_t[(k, q)][:, isl],
                                x_t[k][:, t0:t0 + tw],
                                start=(k == 0), stop=(k == KH - 1))
                        for k in range(KH):
                            nc.tensor.matmul(
                                u_ps, wu_t[(k, q)][:, isl],
                                x_t[k][:, t0:t0 + tw],
                                start=(k == 0), stop=(k == KH - 1))
                        sg = actB.tile([KP, tw], f32, tag="sg",
                                       name=f"sg{i}_{t0}")
                        nc.scalar.activation(
                            sg, g_ps, mybir.ActivationFunctionType.Silu)
                        nc.vector.tensor_mul(
                            h_t[i][:, t0:t0 + tw], sg, u_ps)

            # ---- phase D: yT = h @ wd, output [H, C]
            # All wd tiles are resident (streamed during phase B), so both
            # token tiles run hh-outer; each output block's copy+store is
            # staggered and the kernel tail is only the last (smallest)
            # block.
            with tc.tile_pool(name="yout", bufs=4) as yp, \
                 tc.tile_pool(name="psD", bufs=1, space="PSUM") as psD:
                for ti, (t0, tw) in enumerate(ttiles):
                    for hh in range(HH):
                        y_ps = psD.tile([KP, tw], f32, tag=f"y{hh}",
                                        name=f"y{hh}_{t0}")
                        for i in range(KI):
                            nc.tensor.matmul(
                                y_ps,
                                wd_t[i][:, hh * KP:(hh + 1) * KP],
                                h_t[i][:, t0:t0 + tw],
                                start=(i == 0), stop=(i == KI - 1))
                        yo = yp.tile([KP, tw], f32, tag="yo",
                                     name=f"yo{hh}_{t0}")
                        nc.vector.tensor_copy(yo, y_ps)
                        nc.scalar.dma_start(
                            out=yT[hh * KP:(hh + 1) * KP, t0:t0 + tw],
                            in_=yo)
    _split_multi_waits(nc)
    return nc


CMAX = 1024   # per-run token capacity (bounded by SBUF for the h tiles)


def _prepare(inputs):
    """Host-side routing + weight folding. Returns (in_maps, idx, wts, C)."""
    hs = np.asarray(inputs["hidden_states"], dtype=np.float32)
    rw = np.asarray(inputs["routing_weights"], dtype=np.float32)
    se = np.asarray(inputs["selected_experts"]).astype(np.int64)
    T = hs.shape[0]

    combine = np.zeros((T, E), dtype=np.float32)
    for k in range(se.shape[1]):
        np.add.at(combine, (np.arange(T), se[:, k]), rw[:, k])

    idx = [np.nonzero(combine[:, e])[0] for e in range(E)]
    wts = [combine[idx[e], e] for e in range(E)]
    maxn = max((len(ix) for ix in idx), default=1)
    C = min(max(512, maxn), CMAX)

    gp = np.asarray(inputs["gate_proj"], dtype=np.float32)
    up = np.asarray(inputs["up_proj"], dtype=np.float32)
    dp = np.asarray(inputs["down_proj"], dtype=np.float32)
    gA = np.asarray(inputs["gate_A"], dtype=np.float32)
    gB = np.asarray(inputs["gate_B"], dtype=np.float32)
    uA = np.asarray(inputs["up_A"], dtype=np.float32)
    uB = np.asarray(inputs["up_B"], dtype=np.float32)
    dA = np.asarray(inputs["down_A"], dtype=np.float32)
    dB = np.asarray(inputs["down_B"], dtype=np.float32)

    npdt = BF16
    wmaps = []
    for e in range(E):
        wge = (gp[e] + SCALING * (gA[e] @ gB[e])).astype(npdt)
        wue = (up[e] + SCALING * (uA[e] @ uB[e])).astype(npdt)
        wde = (dp[e] + SCALING * (dA[e] @ dB[e])).astype(npdt)
        wmaps.append({"wg": wge, "wu": wue, "wd": wde})
    return hs, wmaps, idx, wts, C, npdt


def kernel(**inputs):
    _setup_paths()
    from concourse.bass_utils import run_bass_kernel_spmd

    hs, wmaps, idx, wts, C, npdt = _prepare(inputs)

    nc = _cache.get(C)
    if nc is None:
        nc = _build(C)
        _cache[C] = nc

    T = hs.shape[0]
    out = np.zeros((T, H), dtype=np.float32)
    maxn = max((len(ix) for ix in idx), default=1)
    nruns = max(1, -(-maxn // C))
    for r in range(nruns):
        in_maps = []
        for e in range(E):
            sub = idx[e][r * C:(r + 1) * C]
            xTe = np.zeros((H, C), dtype=npdt)
            if len(sub):
                xTe[:, :len(sub)] = hs[sub].T.astype(npdt)
            in_maps.append({"xT": xTe, **wmaps[e]})
        try:
            res = run_bass_kernel_spmd(
                nc, in_maps, core_ids=list(range(NCORES)))
        except Exception:
            import time
            time.sleep(2.0)
            res = run_bass_kernel_spmd(
                nc, in_maps, core_ids=list(range(NCORES)))

        # expose for external profiling harnesses (test.py)
        kernel._last = {"nc": nc, "in_maps": in_maps, "results": res}

        for e in range(E):
            sub = idx[e][r * C:(r + 1) * C]
            if not len(sub):
                continue
            w = wts[e][r * C:(r + 1) * C]
            yTe = res.results[e]["yT"]          # [H, C] fp32
            out[sub] += w[:, None] * yTe[:, :len(sub)].T
    return out


# revision 5
# speedup vs baseline: 1.0049x; 1.0049x over previous
"""MoE + LoRA expert FFN kernel for 8 Trainium2 NeuronCores.

Strategy (expert-parallel, host dispatch/combine):
  - E=8 experts, one expert per core. The host groups tokens by expert
    (a token appears once per distinct selected expert; duplicate
    selections collapse with summed routing weight), pads each group to
    a uniform capacity C, and ships per-core inputs:
        xT  [H, C]   tokens routed to this core's expert, transposed
        wg  [H, I]   gate_proj + 2*gate_A@gate_B   (LoRA folded)
        wu  [H, I]   up_proj   + 2*up_A@up_B
        wd  [I, H]   down_proj + 2*down_A@down_B
    and receives yT [H, C] fp32 = (silu(x@wg) * (x@wu)) @ wd, transposed.
  - Everything on device stays feature-major (features on partitions,
    tokens on the moving free dim) so no transposes are needed.
  - All matmul operands are bf16 (measured end-to-end rel err 4.3e-3 vs
    the 2e-2 gate); PSUM accumulation is fp32. bf16 halves HBM/SBUF
    traffic vs fp32r at the same PE rate.
  - The host scales each token's expert output by its routing weight and
    scatters back into the [T, H] result.

LoRA folding is exact algebra: x@W + s*(x@A)@B == x@(W + s*A@B).

Schedule per core:
  - x tiles ride the two HWDGE rings (sync/scalar), interleaved with the
    first weight group so the phase-B k-accumulation can start as soon
    as the k=0 pair lands (~9.5us) instead of waiting for a full 2MB
    SWDGE x load.
  - A short burst of dummy matmuls warms the PE clock (HAM un-throttle)
    while the first DMAs are in flight.
  - wd streams on gpsimd/SWDGE during phase B (which no longer needs
    gpsimd for x), so phase D runs with all weights resident and its
    final copy+store tail is one small token tile.
"""

import numpy as np
import ml_dtypes

E, H, I, R, TOPK = 8, 1024, 2816, 8, 2
SCALING = 2.0
NCORES = 8
KP = 128          # partition / contraction tile
NTOK = 512        # moving-dim (token) tile
BF16 = ml_dtypes.bfloat16
NDUMMY = 30       # PE-warmup matmuls before the first real one

_cache = {}


def _setup_paths():
    import sys
    for p in ("/opt/trn_rl_repo", "/root/.axon_site"):
        if p not in sys.path:
            sys.path.insert(0, p)


def _split_multi_waits(nc):
    """The walrus in this container accepts at most 1 sem wait per
    instruction (2 on EventSemaphore); Tile emits more. Rewrite each block,
    moving excess waits onto preceding single-wait NoOps on the same
    engine (engines execute in order, so semantics are preserved)."""
    _setup_paths()
    from bass_rust import SyncInfo
    from concourse import mybir

    ctr = [0]
    for f in nc.m.functions:
        for bb in f.blocks:
            insts = bb.instructions
            new = []
            changed = False
            for inst in insts:
                si = inst.sync_info
                waits = list(si.on_wait or []) if si is not None else []
                cap = 2 if isinstance(inst, mybir.InstEventSemaphore) else 1
                if len(waits) > cap:
                    changed = True
                    for w in waits[:-cap]:
                        nop = mybir.InstNoOp(
                            name=f"SW-{ctr[0]}", ins=[], outs=[])
                        ctr[0] += 1
                        nop.engine = inst.engine
                        nop.sync_info = SyncInfo(on_wait=[w], on_update=[])
                        new.append(nop)
                    inst.sync_info = SyncInfo(
                        on_wait=waits[-cap:],
                        on_update=list(si.on_update or []))
                new.append(inst)
            if changed:
                bb.instructions = new


def _token_tiles(C):
    tiles = []
    t0 = 0
    while t0 < C:
        tw = min(NTOK, C - t0)
        tiles.append((t0, tw))
        t0 += tw
    return tiles


def _build(C):
    """Build the per-core Bass program for token capacity C."""
    _setup_paths()
    import concourse.bass as bass
    import concourse.tile as tile
    from concourse import mybir

    f32 = mybir.dt.float32
    sdt = mybir.dt.bfloat16

    KH = H // KP            # 8 contraction chunks over H
    KI = I // KP            # 22 chunks over I
    HH = H // KP            # 8 output row blocks

    nc = bass.Bass("TRN2", target_bir_lowering=False, debug=False,
                   num_devices=NCORES)
    xT = nc.declare_dram_parameter("xT", [H, C], sdt, isOutput=False)
    wg = nc.declare_dram_parameter("wg", [H, I], sdt, isOutput=False)
    wu = nc.declare_dram_parameter("wu", [H, I], sdt, isOutput=False)
    wd = nc.declare_dram_parameter("wd", [I, H], sdt, isOutput=False)
    yT = nc.declare_dram_parameter("yT", [H, C], f32, isOutput=True)

    ttiles = _token_tiles(C)

    # ramped weight column groups (in i-tiles): small first for fast start
    groups = [2, 2, 2]
    while sum(groups) < KI:
        groups.append(min(4, KI - sum(groups)))
    gstart = [sum(groups[:j]) for j in range(len(groups))]
    i2q = {}
    for qq, (g0, gn) in enumerate(zip(gstart, groups)):
        for i in range(g0, g0 + gn):
            i2q[i] = (qq, i - g0)

    with tile.TileContext(nc) as tc:
        with tc.tile_pool(name="hh", bufs=1) as hp, \
             tc.tile_pool(name="wdp", bufs=1) as wdp:
            h_t = [hp.tile([KP, C], sdt, tag=f"h{i}", name=f"h{i}")
                   for i in range(KI)]
            wd_t = {}

            # ---- phase B: h = silu(x@wg) * (x@wu), feature-major [I, C]
            with tc.tile_pool(name="xp", bufs=1) as xp, \
                 tc.tile_pool(name="wst", bufs=32) as wst, \
                 tc.tile_pool(name="psB", bufs=4, space="PSUM") as psB, \
                 tc.tile_pool(name="actB", bufs=4) as actB:
                # PE warm-up: dummy matmuls un-throttle the HAM while the
                # first DMAs are in flight (must be emitted before any
                # dma-dependent matmul on the PE queue)
                wsrc = actB.tile([KP, 256], mybir.dt.bfloat16,
                                 tag="wsrc", name="wsrc")
                nc.vector.memset(wsrc, 0.0)
                wdst = psB.tile([KP, 256], f32, tag="g", name="wdst")
                for w in range(NDUMMY):
                    nc.tensor.matmul(wdst, wsrc[:, :128], wsrc,
                                     start=(w == 0), stop=(w == NDUMMY - 1))

                # x split by token halves: the first halves (all tt=0
                # groups need them) ride the two HWDGE rings pair-
                # interleaved with group-0 weights so the first PSUM
                # group's k-accumulation is paced by pair arrivals; the
                # second halves ride SWDGE (needed ~7us later, at the
                # first tt=1 group).
                wg_t, wu_t = {}, {}
                NG = len(groups)
                XH0 = min(NTOK, C)

                def load_w_one(mat, store, k, q, eng):
                    c0 = gstart[q] * KP
                    cw = groups[q] * KP
                    t = wst.tile([KP, 4 * KP], sdt, tag="w",
                                 name=f"w{'gu'[mat]}{k}_{q}")
                    eng.dma_start(
                        out=t[:, :cw],
                        in_=(wg if mat == 0 else wu)[
                            k * KP:(k + 1) * KP, c0:c0 + cw])
                    store[(k, q)] = t

                x_t = [xp.tile([KP, C], sdt, tag=f"x{k}", name=f"x{k}")
                       for k in range(KH)]
                for k in range(KH):
                    eng = nc.sync if (k % 2 == 0) else nc.scalar
                    eng.dma_start(out=x_t[k][:, :XH0],
                                  in_=xT[k * KP:(k + 1) * KP, :XH0])
                    load_w_one(0, wg_t, k, 0, nc.sync)
                    load_w_one(1, wu_t, k, 0, nc.scalar)
                if XH0 < C:
                    for k in range(KH):
                        nc.gpsimd.dma_start(
                            out=x_t[k][:, XH0:],
                            in_=xT[k * KP:(k + 1) * KP, XH0:])

                def load_w_group(q, mats=(0, 1)):
                    # allocation order must track consumption order — the
                    # shared-tag slot pool recycles FIFO
                    for k in range(KH):
                        if 0 in mats:
                            load_w_one(0, wg_t, k, q, nc.sync)
                        if 1 in mats:
                            load_w_one(1, wu_t, k, q, nc.scalar)

                load_w_group(1)
                # wd streaming on gpsimd/SWDGE: 3 tiles up-front, the rest
                # staggered through the i-loop so all of wd is resident
                # well before phase D starts.
                for i in range(3):
                    t = wdp.tile([KP, H], sdt, tag=f"wds{i}",
                                 name=f"wds{i}")
                    nc.gpsimd.dma_start(
                        out=t, in_=wd[i * KP:(i + 1) * KP, :])
                    wd_t[i] = t

                for i in range(KI):
                    q, r = i2q[i]
                    if r == 0 and q + 2 < NG:
                        load_w_group(q + 2)
                    # stagger one wd tile per i-tile
                    wdi = i + 3
                    if wdi < KI:
                        t = wdp.tile([KP, H], sdt, tag=f"wds{wdi}",
                                     name=f"wds{wdi}")
                        nc.gpsimd.dma_start(
                            out=t, in_=wd[wdi * KP:(wdi + 1) * KP, :])
                        wd_t[wdi] = t
                    isl = slice(r * KP, (r + 1) * KP)
                    for ti, (t0, tw) in enumerate(ttiles):
                        g_ps = psB.tile([KP, tw], f32, tag="g",
                                        name=f"g{i}_{t0}")
                        u_ps = psB.tile([KP, tw], f32, tag="u",
                                        name=f"u{i}_{t0}")
                        for k in range(KH):
                            nc.tensor.matmul(
                                g_ps, wg_t[(k, q)][:, isl],
                                x_t[k][:, t0:t0 + tw],
                                start=(k == 0), stop=(k == KH - 1))
                        for k in range(KH):
                            nc.tensor.matmul(
                                u_ps, wu_t[(k, q)][:, isl],
                                x_t[k][:, t0:t0 + tw],
                                start=(k == 0), stop=(k == KH - 1))
                        sg = actB.tile([KP, tw], f32, tag="sg",
                                       name=f"sg{i}_{t0}")
                        nc.scalar.activation(
                            sg, g_ps, mybir.ActivationFunctionType.Silu)
                        nc.vector.tensor_mul(
                            h_t[i][:, t0:t0 + tw], sg, u_ps)

            # ---- phase D: yT = h @ wd, output [H, C]
            # All wd tiles are resident (streamed during phase B), so both
            # token tiles run hh-outer; each output block's copy+store is
            # staggered and the kernel tail is only the last (smallest)
            # block.
            with tc.tile_pool(name="yout", bufs=4) as yp, \
                 tc.tile_pool(name="psD", bufs=1, space="PSUM") as psD:
                for ti, (t0, tw) in enumerate(ttiles):
                    for hh in range(HH):
                        y_ps = psD.tile([KP, tw], f32, tag=f"y{hh}",
                                        name=f"y{hh}_{t0}")
                        for i in range(KI):
                            nc.tensor.matmul(
                                y_ps,
                                wd_t[i][:, hh * KP:(hh + 1) * KP],
                                h_t[i][:, t0:t0 + tw],
                                start=(i == 0), stop=(i == KI - 1))
                        yo = yp.tile([KP, tw], f32, tag="yo",
                                     name=f"yo{hh}_{t0}")
                        nc.vector.tensor_copy(yo, y_ps)
                        nc.scalar.dma_start(
                            out=yT[hh * KP:(hh + 1) * KP, t0:t0 + tw],
                            in_=yo)
    _split_multi_waits(nc)
    return nc


CMAX = 1024   # per-run token capacity (bounded by SBUF for the h tiles)


def _prepare(inputs):
    """Host-side routing + weight folding. Returns (in_maps, idx, wts, C)."""
    hs = np.asarray(inputs["hidden_states"], dtype=np.float32)
    rw = np.asarray(inputs["routing_weights"], dtype=np.float32)
    se = np.asarray(inputs["selected_experts"]).astype(np.int64)
    T = hs.shape[0]

    combine = np.zeros((T, E), dtype=np.float32)
    for k in range(se.shape[1]):
        np.add.at(combine, (np.arange(T), se[:, k]), rw[:, k])

    idx = [np.nonzero(combine[:, e])[0] for e in range(E)]
    wts = [combine[idx[e], e] for e in range(E)]
    maxn = max((len(ix) for ix in idx), default=1)
    C = min(max(512, maxn), CMAX)

    gp = np.asarray(inputs["gate_proj"], dtype=np.float32)
    up = np.asarray(inputs["up_proj"], dtype=np.float32)
    dp = np.asarray(inputs["down_proj"], dtype=np.float32)
    gA = np.asarray(inputs["gate_A"], dtype=np.float32)
    gB = np.asarray(inputs["gate_B"], dtype=np.float32)
    uA = np.asarray(inputs["up_A"], dtype=np.float32)
    uB = np.asarray(inputs["up_B"], dtype=np.float32)
    dA = np.asarray(inputs["down_A"], dtype=np.float32)
    dB = np.asarray(inputs["down_B"], dtype=np.float32)

    npdt = BF16
    wmaps = []
    for e in range(E):
        wge = (gp[e] + SCALING * (gA[e] @ gB[e])).astype(npdt)
        wue = (up[e] + SCALING * (uA[e] @ uB[e])).astype(npdt)
        wde = (dp[e] + SCALING * (dA[e] @ dB[e])).astype(npdt)
        wmaps.append({"wg": wge, "wu": wue, "wd": wde})
    return hs, wmaps, idx, wts, C, npdt


def kernel(**inputs):
    _setup_paths()
    from concourse.bass_utils import run_bass_kernel_spmd

    hs, wmaps, idx, wts, C, npdt = _prepare(inputs)

    nc = _cache.get(C)
    if nc is None:
        nc = _build(C)
        _cache[C] = nc

    T = hs.shape[0]
    out = np.zeros((T, H), dtype=np.float32)
    maxn = max((len(ix) for ix in idx), default=1)
    nruns = max(1, -(-maxn // C))
    for r in range(nruns):
        in_maps = []
        for e in range(E):
            sub = idx[e][r * C:(r + 1) * C]
            xTe = np.zeros((H, C), dtype=npdt)
            if len(sub):
                xTe[:, :len(sub)] = hs[sub].T.astype(npdt)
            in_maps.append({"xT": xTe, **wmaps[e]})
        try:
            res = run_bass_kernel_spmd(
                nc, in_maps, core_ids=list(range(NCORES)))
        except Exception:
            import time
            time.sleep(2.0)
            res = run_bass_kernel_spmd(
                nc, in_maps, core_ids=list(range(NCORES)))

        # expose for external profiling harnesses (test.py)
        kernel._last = {"nc": nc, "in_maps": in_maps, "results": res}

        for e in range(E):
            sub = idx[e][r * C:(r + 1) * C]
            if not len(sub):
                continue
            w = wts[e][r * C:(r + 1) * C]
            yTe = res.results[e]["yT"]          # [H, C] fp32
            out[sub] += w[:, None] * yTe[:, :len(sub)].T
    return out


# revision 7
# speedup vs baseline: 1.0064x; 1.0015x over previous
"""MoE + LoRA expert FFN kernel for 8 Trainium2 NeuronCores.

Strategy (expert-parallel, host dispatch/combine):
  - E=8 experts, one expert per core. The host groups tokens by expert
    (a token appears once per distinct selected expert; duplicate
    selections collapse with summed routing weight), pads each group to
    a uniform capacity C, and ships per-core inputs:
        xT  [H, C]   tokens routed to this core's expert, transposed
        wg  [H, I]   gate_proj + 2*gate_A@gate_B   (LoRA folded)
        wu  [H, I]   up_proj   + 2*up_A@up_B
        wd  [I, H]   down_proj + 2*down_A@down_B
    and receives yT [H, C] fp32 = (silu(x@wg) * (x@wu)) @ wd, transposed.
  - Everything on device stays feature-major (features on partitions,
    tokens on the moving free dim) so no transposes are needed.
  - All matmul operands are bf16 (measured end-to-end rel err 4.3e-3 vs
    the 2e-2 gate); PSUM accumulation is fp32. bf16 halves HBM/SBUF
    traffic vs fp32r at the same PE rate.
  - The host scales each token's expert output by its routing weight and
    scatters back into the [T, H] result.

LoRA folding is exact algebra: x@W + s*(x@A)@B == x@(W + s*A@B).

Schedule per core:
  - x tiles ride the two HWDGE rings (sync/scalar), interleaved with the
    first weight group so the phase-B k-accumulation can start as soon
    as the k=0 pair lands (~9.5us) instead of waiting for a full 2MB
    SWDGE x load.
  - A short burst of dummy matmuls warms the PE clock (HAM un-throttle)
    while the first DMAs are in flight.
  - wd streams on gpsimd/SWDGE during phase B (which no longer needs
    gpsimd for x), so phase D runs with all weights resident and its
    final copy+store tail is one small token tile.
"""

import numpy as np
import ml_dtypes

E, H, I, R, TOPK = 8, 1024, 2816, 8, 2
SCALING = 2.0
NCORES = 8
KP = 128          # partition / contraction tile
NTOK = 512        # moving-dim (token) tile
BF16 = ml_dtypes.bfloat16
NDUMMY = 12       # PE-warmup matmuls before the first real one

_cache = {}


def _setup_paths():
    import sys
    for p in ("/opt/trn_rl_repo", "/root/.axon_site"):
        if p not in sys.path:
            sys.path.insert(0, p)


def _split_multi_waits(nc):
    """The walrus in this container accepts at most 1 sem wait per
    instruction (2 on EventSemaphore); Tile emits more. Rewrite each block,
    moving excess waits onto preceding single-wait NoOps on the same
    engine (engines execute in order, so semantics are preserved)."""
    _setup_paths()
    from bass_rust import SyncInfo
    from concourse import mybir

    ctr = [0]
    for f in nc.m.functions:
        for bb in f.blocks:
            insts = bb.instructions
            new = []
            changed = False
            for inst in insts:
                si = inst.sync_info
                waits = list(si.on_wait or []) if si is not None else []
                cap = 2 if isinstance(inst, mybir.InstEventSemaphore) else 1
                if len(waits) > cap:
                    changed = True
                    for w in waits[:-cap]:
                        nop = mybir.InstNoOp(
                            name=f"SW-{ctr[0]}", ins=[], outs=[])
                        ctr[0] += 1
                        nop.engine = inst.engine
                        nop.sync_info = SyncInfo(on_wait=[w], on_update=[])
                        new.append(nop)
                    inst.sync_info = SyncInfo(
                        on_wait=waits[-cap:],
                        on_update=list(si.on_update or []))
                new.append(inst)
            if changed:
                bb.instructions = new


def _token_tiles(C):
    tiles = []
    t0 = 0
    while t0 < C:
        tw = min(NTOK, C - t0)
        tiles.append((t0, tw))
        t0 += tw
    return tiles


def _build(C):
    """Build the per-core Bass program for token capacity C."""
    _setup_paths()
    import concourse.bass as bass
    import concourse.tile as tile
    from concourse import mybir

    f32 = mybir.dt.float32
    sdt = mybir.dt.bfloat16

    KH = H // KP            # 8 contraction chunks over H
    KI = I // KP            # 22 chunks over I
    HH = H // KP            # 8 output row blocks

    nc = bass.Bass("TRN2", target_bir_lowering=False, debug=False,
                   num_devices=NCORES)
    xT = nc.declare_dram_parameter("xT", [H, C], sdt, isOutput=False)
    wg = nc.declare_dram_parameter("wg", [H, I], sdt, isOutput=False)
    wu = nc.declare_dram_parameter("wu", [H, I], sdt, isOutput=False)
    wd = nc.declare_dram_parameter("wd", [I, H], sdt, isOutput=False)
    yT = nc.declare_dram_parameter("yT", [H, C], f32, isOutput=True)

    ttiles = _token_tiles(C)

    # ramped weight column groups (in i-tiles): small first for fast start
    groups = [1, 1, 2, 2]
    while sum(groups) < KI:
        groups.append(min(4, KI - sum(groups)))
    gstart = [sum(groups[:j]) for j in range(len(groups))]
    i2q = {}
    for qq, (g0, gn) in enumerate(zip(gstart, groups)):
        for i in range(g0, g0 + gn):
            i2q[i] = (qq, i - g0)

    with tile.TileContext(nc) as tc:
        with tc.tile_pool(name="hh", bufs=1) as hp, \
             tc.tile_pool(name="wdp", bufs=1) as wdp:
            h_t = [hp.tile([KP, C], sdt, tag=f"h{i}", name=f"h{i}")
                   for i in range(KI)]
            wd_t = {}

            # ---- phase B: h = silu(x@wg) * (x@wu), feature-major [I, C]
            with tc.tile_pool(name="xp", bufs=1) as xp, \
                 tc.tile_pool(name="wst", bufs=32) as wst, \
                 tc.tile_pool(name="psB", bufs=4, space="PSUM") as psB, \
                 tc.tile_pool(name="actB", bufs=4) as actB:
                # PE warm-up: dummy matmuls un-throttle the HAM while the
                # first DMAs are in flight (must be emitted before any
                # dma-dependent matmul on the PE queue)
                wsrc = actB.tile([KP, 256], mybir.dt.bfloat16,
                                 tag="wsrc", name="wsrc")
                nc.vector.memset(wsrc, 0.0)
                wdst = psB.tile([KP, 256], f32, tag="g", name="wdst")
                for w in range(NDUMMY):
                    nc.tensor.matmul(wdst, wsrc[:, :128], wsrc,
                                     start=(w == 0), stop=(w == NDUMMY - 1))

                # x split by token halves: the first halves (all tt=0
                # groups need them) ride the two HWDGE rings pair-
                # interleaved with group-0 weights so the first PSUM
                # group's k-accumulation is paced by pair arrivals; the
                # second halves ride SWDGE (needed ~7us later, at the
                # first tt=1 group).
                wg_t, wu_t = {}, {}
                NG = len(groups)
                XH0 = min(NTOK, C)

                def load_w_one(mat, store, k, q, eng):
                    c0 = gstart[q] * KP
                    cw = groups[q] * KP
                    t = wst.tile([KP, 4 * KP], sdt, tag="w",
                                 name=f"w{'gu'[mat]}{k}_{q}")
                    eng.dma_start(
                        out=t[:, :cw],
                        in_=(wg if mat == 0 else wu)[
                            k * KP:(k + 1) * KP, c0:c0 + cw])
                    store[(k, q)] = t

                x_t = [xp.tile([KP, C], sdt, tag=f"x{k}", name=f"x{k}")
                       for k in range(KH)]
                for k in range(KH):
                    eng = nc.sync if (k % 2 == 0) else nc.scalar
                    eng.dma_start(out=x_t[k][:, :XH0],
                                  in_=xT[k * KP:(k + 1) * KP, :XH0])
                    load_w_one(0, wg_t, k, 0, nc.sync)
                    load_w_one(1, wu_t, k, 0, nc.scalar)
                if XH0 < C:
                    for k in range(KH):
                        nc.gpsimd.dma_start(
                            out=x_t[k][:, XH0:],
                            in_=xT[k * KP:(k + 1) * KP, XH0:])

                def load_w_group(q, mats=(0, 1)):
                    # allocation order must track consumption order — the
                    # shared-tag slot pool recycles FIFO
                    for k in range(KH):
                        if 0 in mats:
                            load_w_one(0, wg_t, k, q, nc.sync)
                        if 1 in mats:
                            load_w_one(1, wu_t, k, q, nc.scalar)

                load_w_group(1)
                # wd streaming on gpsimd/SWDGE: 3 tiles up-front, the rest
                # staggered through the i-loop so all of wd is resident
                # well before phase D starts.
                for i in range(3):
                    t = wdp.tile([KP, H], sdt, tag=f"wds{i}",
                                 name=f"wds{i}")
                    nc.gpsimd.dma_start(
                        out=t, in_=wd[i * KP:(i + 1) * KP, :])
                    wd_t[i] = t

                for i in range(KI):
                    q, r = i2q[i]
                    if r == 0 and q + 2 < NG:
                        load_w_group(q + 2)
                    # stagger one wd tile per i-tile
                    wdi = i + 3
                    if wdi < KI:
                        t = wdp.tile([KP, H], sdt, tag=f"wds{wdi}",
                                     name=f"wds{wdi}")
                        nc.gpsimd.dma_start(
                            out=t, in_=wd[wdi * KP:(wdi + 1) * KP, :])
                        wd_t[wdi] = t
                    isl = slice(r * KP, (r + 1) * KP)
                    for ti, (t0, tw) in enumerate(ttiles):
                        g_ps = psB.tile([KP, tw], f32, tag="g",
                                        name=f"g{i}_{t0}")
                        u_ps = psB.tile([KP, tw], f32, tag="u",
                                        name=f"u{i}_{t0}")
                        for k in range(KH):
                            nc.tensor.matmul(
                                g_ps, wg_t[(k, q)][:, isl],
                                x_t[k][:, t0:t0 + tw],
                                start=(k == 0), stop=(k == KH - 1))
                        for k in range(KH):
                            nc.tensor.matmul(
                                u_ps, wu_t[(k, q)][:, isl],
                                x_t[k][:, t0:t0 + tw],
                                start=(k == 0), stop=(k == KH - 1))
                        sg = actB.tile([KP, tw], f32, tag="sg",
                                       name=f"sg{i}_{t0}")
                        nc.scalar.activation(
                            sg, g_ps, mybir.ActivationFunctionType.Silu)
                        nc.vector.tensor_mul(
                            h_t[i][:, t0:t0 + tw], sg, u_ps)

            # ---- phase D: yT = h @ wd, output [H, C]
            # All wd tiles are resident (streamed during phase B), so both
            # token tiles run hh-outer; each output block's copy+store is
            # staggered and the kernel tail is only the last (smallest)
            # block.
            with tc.tile_pool(name="yout", bufs=4) as yp, \
                 tc.tile_pool(name="psD", bufs=1, space="PSUM") as psD:
                for ti, (t0, tw) in enumerate(ttiles):
                    for hh in range(HH):
                        y_ps = psD.tile([KP, tw], f32, tag=f"y{hh}",
                                        name=f"y{hh}_{t0}")
                        for i in range(KI):
                            nc.tensor.matmul(
                                y_ps,
                                wd_t[i][:, hh * KP:(hh + 1) * KP],
                                h_t[i][:, t0:t0 + tw],
                                start=(i == 0), stop=(i == KI - 1))
                        yo = yp.tile([KP, tw], f32, tag="yo",
                                     name=f"yo{hh}_{t0}")
                        nc.vector.tensor_copy(yo, y_ps)
                        nc.scalar.dma_start(
                            out=yT[hh * KP:(hh + 1) * KP, t0:t0 + tw],
                            in_=yo)
    _split_multi_waits(nc)
    return nc


CMAX = 1024   # per-run token capacity (bounded by SBUF for the h tiles)


def _prepare(inputs):
    """Host-side routing + weight folding. Returns (in_maps, idx, wts, C)."""
    hs = np.asarray(inputs["hidden_states"], dtype=np.float32)
    rw = np.asarray(inputs["routing_weights"], dtype=np.float32)
    se = np.asarray(inputs["selected_experts"]).astype(np.int64)
    T = hs.shape[0]

    combine = np.zeros((T, E), dtype=np.float32)
    for k in range(se.shape[1]):
        np.add.at(combine, (np.arange(T), se[:, k]), rw[:, k])

    idx = [np.nonzero(combine[:, e])[0] for e in range(E)]
    wts = [combine[idx[e], e] for e in range(E)]
    maxn = max((len(ix) for ix in idx), default=1)
    C = min(max(512, maxn), CMAX)

    gp = np.asarray(inputs["gate_proj"], dtype=np.float32)
    up = np.asarray(inputs["up_proj"], dtype=np.float32)
    dp = np.asarray(inputs["down_proj"], dtype=np.float32)
    gA = np.asarray(inputs["gate_A"], dtype=np.float32)
    gB = np.asarray(inputs["gate_B"], dtype=np.float32)
    uA = np.asarray(inputs["up_A"], dtype=np.float32)
    uB = np.asarray(inputs["up_B"], dtype=np.float32)
    dA = np.asarray(inputs["down_A"], dtype=np.float32)
    dB = np.asarray(inputs["down_B"], dtype=np.float32)

    npdt = BF16
    wmaps = []
    for e in range(E):
        wge = (gp[e] + SCALING * (gA[e] @ gB[e])).astype(npdt)
        wue = (up[e] + SCALING * (uA[e] @ uB[e])).astype(npdt)
        wde = (dp[e] + SCALING * (dA[e] @ dB[e])).astype(npdt)
        wmaps.append({"wg": wge, "wu": wue, "wd": wde})
    return hs, wmaps, idx, wts, C, npdt


def kernel(**inputs):
    _setup_paths()
    from concourse.bass_utils import run_bass_kernel_spmd

    hs, wmaps, idx, wts, C, npdt = _prepare(inputs)

    nc = _cache.get(C)
    if nc is None:
        nc = _build(C)
        _cache[C] = nc

    T = hs.shape[0]
    out = np.zeros((T, H), dtype=np.float32)
    maxn = max((len(ix) for ix in idx), default=1)
    nruns = max(1, -(-maxn // C))
    for r in range(nruns):
        in_maps = []
        for e in range(E):
            sub = idx[e][r * C:(r + 1) * C]
            xTe = np.zeros((H, C), dtype=npdt)
            if len(sub):
                xTe[:, :len(sub)] = hs[sub].T.astype(npdt)
            in_maps.append({"xT": xTe, **wmaps[e]})
        try:
            res = run_bass_kernel_spmd(
                nc, in_maps, core_ids=list(range(NCORES)))
        except Exception:
            import time
            time.sleep(2.0)
            res = run_bass_kernel_spmd(
                nc, in_maps, core_ids=list(range(NCORES)))

        # expose for external profiling harnesses (test.py)
        kernel._last = {"nc": nc, "in_maps": in_maps, "results": res}

        for e in range(E):
            sub = idx[e][r * C:(r + 1) * C]
            if not len(sub):
                continue
            w = wts[e][r * C:(r + 1) * C]
            yTe = res.results[e]["yT"]          # [H, C] fp32
            out[sub] += w[:, None] * yTe[:, :len(sub)].T
    return out


# revision 11
# speedup vs baseline: 1.0337x; 1.0271x over previous
"""MoE + LoRA expert FFN kernel for 8 Trainium2 NeuronCores.

Strategy (expert-parallel, host dispatch/combine):
  - E=8 experts, one expert per core. The host groups tokens by expert
    (a token appears once per distinct selected expert; duplicate
    selections collapse with summed routing weight), pads each group to
    a uniform capacity C, and ships per-core inputs:
        xT  [H, C]   tokens routed to this core's expert, transposed
        wg  [H, I]   gate_proj + 2*gate_A@gate_B   (LoRA folded)
        wu  [H, I]   up_proj   + 2*up_A@up_B
        wd  [I, H]   down_proj + 2*down_A@down_B
    and receives yT [H, C] fp32 = (silu(x@wg) * (x@wu)) @ wd, transposed.
  - Everything on device stays feature-major (features on partitions,
    tokens on the moving free dim) so no transposes are needed.
  - All matmul operands are bf16 (measured end-to-end rel err 4.3e-3 vs
    the 2e-2 gate); PSUM accumulation is fp32. bf16 halves HBM/SBUF
    traffic vs fp32r at the same PE rate.
  - The host scales each token's expert output by its routing weight and
    scatters back into the [T, H] result.

LoRA folding is exact algebra: x@W + s*(x@A)@B == x@(W + s*A@B).

Schedule per core:
  - x tiles ride the two HWDGE rings (sync/scalar), interleaved with the
    first weight group so the phase-B k-accumulation can start as soon
    as the k=0 pair lands (~9.5us) instead of waiting for a full 2MB
    SWDGE x load.
  - A short burst of dummy matmuls warms the PE clock (HAM un-throttle)
    while the first DMAs are in flight.
  - wd streams on gpsimd/SWDGE during phase B (which no longer needs
    gpsimd for x), so phase D runs with all weights resident and its
    final copy+store tail is one small token tile.
"""

import numpy as np
import ml_dtypes

E, H, I, R, TOPK = 8, 1024, 2816, 8, 2
SCALING = 2.0
NCORES = 8
KP = 128          # partition / contraction tile
NTOK = 512        # moving-dim (token) tile
BF16 = ml_dtypes.bfloat16
NDUMMY = 12       # PE-warmup matmuls before the first real one

_cache = {}


def _setup_paths():
    import sys
    for p in ("/opt/trn_rl_repo", "/root/.axon_site"):
        if p not in sys.path:
            sys.path.insert(0, p)


def _split_multi_waits(nc):
    """The walrus in this container accepts at most 1 sem wait per
    instruction (2 on EventSemaphore); Tile emits more. Rewrite each block,
    moving excess waits onto preceding single-wait NoOps on the same
    engine (engines execute in order, so semantics are preserved)."""
    _setup_paths()
    from bass_rust import SyncInfo
    from concourse import mybir

    ctr = [0]
    for f in nc.m.functions:
        for bb in f.blocks:
            insts = bb.instructions
            new = []
            changed = False
            for inst in insts:
                si = inst.sync_info
                waits = list(si.on_wait or []) if si is not None else []
                cap = 2 if isinstance(inst, mybir.InstEventSemaphore) else 1
                if len(waits) > cap:
                    changed = True
                    for w in waits[:-cap]:
                        nop = mybir.InstNoOp(
                            name=f"SW-{ctr[0]}", ins=[], outs=[])
                        ctr[0] += 1
                        nop.engine = inst.engine
                        nop.sync_info = SyncInfo(on_wait=[w], on_update=[])
                        new.append(nop)
                    inst.sync_info = SyncInfo(
                        on_wait=waits[-cap:],
                        on_update=list(si.on_update or []))
                new.append(inst)
            if changed:
                bb.instructions = new


def _token_tiles(C):
    tiles = []
    t0 = 0
    while t0 < C:
        tw = min(NTOK, C - t0)
        tiles.append((t0, tw))
        t0 += tw
    return tiles


def _build(C):
    """Build the per-core Bass program for token capacity C."""
    _setup_paths()
    import concourse.bass as bass
    import concourse.tile as tile
    from concourse import mybir

    f32 = mybir.dt.float32
    sdt = mybir.dt.bfloat16

    KH = H // KP            # 8 contraction chunks over H
    KI = I // KP            # 22 chunks over I
    HH = H // KP            # 8 output row blocks

    nc = bass.Bass("TRN2", target_bir_lowering=False, debug=False,
                   num_devices=NCORES)
    xT = nc.declare_dram_parameter("xT", [H, C], sdt, isOutput=False)
    wg = nc.declare_dram_parameter("wg", [H, I], sdt, isOutput=False)
    wu = nc.declare_dram_parameter("wu", [H, I], sdt, isOutput=False)
    wd = nc.declare_dram_parameter("wd", [I, H], sdt, isOutput=False)
    yT = nc.declare_dram_parameter("yT", [H, C], f32, isOutput=True)

    ttiles = _token_tiles(C)

    # weight column groups (in i-tiles); group 0 feeds the k-wavefront
    groups = [4]
    while sum(groups) < KI:
        groups.append(min(4, KI - sum(groups)))
    gstart = [sum(groups[:j]) for j in range(len(groups))]
    i2q = {}
    for qq, (g0, gn) in enumerate(zip(gstart, groups)):
        for i in range(g0, g0 + gn):
            i2q[i] = (qq, i - g0)

    with tile.TileContext(nc) as tc:
        with tc.tile_pool(name="hh", bufs=1) as hp, \
             tc.tile_pool(name="wdp", bufs=1) as wdp:
            h_t = [hp.tile([KP, C], sdt, tag=f"h{i}", name=f"h{i}")
                   for i in range(KI)]
            wd_t = {}

            # ---- phase B: h = silu(x@wg) * (x@wu), feature-major [I, C]
            with tc.tile_pool(name="xp", bufs=1) as xp, \
                 tc.tile_pool(name="wst", bufs=32) as wst, \
                 tc.tile_pool(name="psB", bufs=4, space="PSUM") as psB, \
                 tc.tile_pool(name="actB", bufs=4) as actB:
                # PE warm-up: dummy matmuls un-throttle the HAM while the
                # first DMAs are in flight (must be emitted before any
                # dma-dependent matmul on the PE queue)
                wsrc = actB.tile([KP, 256], mybir.dt.bfloat16,
                                 tag="wsrc", name="wsrc")
                nc.vector.memset(wsrc, 0.0)
                # dummy PSUM tile shares the g tag ring (full slot size so
                # the ring geometry matches the real g tiles)
                wdst = psB.tile([KP, NTOK], f32, tag="g", name="wdst")
                for w in range(NDUMMY):
                    nc.tensor.matmul(wdst[:, :256], wsrc[:, :128], wsrc,
                                     start=(w == 0), stop=(w == NDUMMY - 1))

                # Front: whole-x tiles and the 4-wide group-0 wg tiles are
                # pair-interleaved on the two HWDGE rings (even k on sync,
                # odd on scalar); group-0 wu rides gpsimd/SWDGE. The first
                # 4 i-tiles then run as a k-major wavefront over 4
                # concurrent PSUM groups, so every (x_k, wg_k) arrival
                # unlocks 4 matmuls.
                wg_t, wu_t = {}, {}
                NG = len(groups)

                def load_w_one(mat, store, k, q, eng):
                    c0 = gstart[q] * KP
                    cw = groups[q] * KP
                    t = wst.tile([KP, 4 * KP], sdt, tag="w",
                                 name=f"w{'gu'[mat]}{k}_{q}")
                    eng.dma_start(
                        out=t[:, :cw],
                        in_=(wg if mat == 0 else wu)[
                            k * KP:(k + 1) * KP, c0:c0 + cw])
                    store[(k, q)] = t

                x_t = [xp.tile([KP, C], sdt, tag=f"x{k}", name=f"x{k}")
                       for k in range(KH)]
                for k in range(KH):
                    eng = nc.sync if (k % 2 == 0) else nc.scalar
                    eng.dma_start(out=x_t[k],
                                  in_=xT[k * KP:(k + 1) * KP, :])
                    load_w_one(0, wg_t, k, 0, eng)
                for k in range(KH):
                    load_w_one(1, wu_t, k, 0, nc.gpsimd)

                def load_w_group(q, mats=(0, 1)):
                    # allocation order must track consumption order — the
                    # shared-tag slot pool recycles FIFO
                    for k in range(KH):
                        if 0 in mats:
                            load_w_one(0, wg_t, k, q, nc.sync)
                        if 1 in mats:
                            load_w_one(1, wu_t, k, q, nc.scalar)

                load_w_group(1)
                # group 2 is loaded at stage 1b: together with groups 0-1
                # it would exceed the 32-slot weight pool and block the
                # scalar queue ahead of the wavefront silus.
                # wd streaming on gpsimd/SWDGE behind group-0 wu, staggered
                # through the i-loop so all of wd is resident well before
                # phase D starts.
                def load_wd(i):
                    t = wdp.tile([KP, H], sdt, tag=f"wds{i}",
                                 name=f"wds{i}")
                    nc.gpsimd.dma_start(
                        out=t, in_=wd[i * KP:(i + 1) * KP, :])
                    wd_t[i] = t

                for i in range(3):
                    load_wd(i)

                WAVE = groups[0]   # 4 i-tiles in the wavefront

                def bgroup(i, tt_list, q, r):
                    isl = slice(r * KP, (r + 1) * KP)
                    for ti, (t0, tw) in enumerate(tt_list):
                        g_ps = psB.tile([KP, tw], f32, tag="g",
                                        name=f"g{i}_{t0}")
                        u_ps = psB.tile([KP, tw], f32, tag="u",
                                        name=f"u{i}_{t0}")
                        for k in range(KH):
                            nc.tensor.matmul(
                                g_ps, wg_t[(k, q)][:, isl],
                                x_t[k][:, t0:t0 + tw],
                                start=(k == 0), stop=(k == KH - 1))
                        for k in range(KH):
                            nc.tensor.matmul(
                                u_ps, wu_t[(k, q)][:, isl],
                                x_t[k][:, t0:t0 + tw],
                                start=(k == 0), stop=(k == KH - 1))
                        sg = actB.tile([KP, tw], f32, tag="sg",
                                       name=f"sg{i}_{t0}")
                        nc.scalar.activation(
                            sg, g_ps, mybir.ActivationFunctionType.Silu)
                        nc.vector.tensor_mul(
                            h_t[i][:, t0:t0 + tw], sg, u_ps)

                # stage 1: k-major wavefront over i=0..WAVE-1, first token
                # tile only (4 g groups + 4 u groups = all 8 PSUM banks)
                t0, tw = ttiles[0]
                gw = [psB.tile([KP, tw], f32, tag="g", name=f"g{i}_{t0}")
                      for i in range(WAVE)]
                uw = [psB.tile([KP, tw], f32, tag="u", name=f"u{i}_{t0}")
                      for i in range(WAVE)]
                for k in range(KH):
                    for i in range(WAVE):
                        nc.tensor.matmul(
                            gw[i], wg_t[(k, 0)][:, i * KP:(i + 1) * KP],
                            x_t[k][:, t0:t0 + tw],
                            start=(k == 0), stop=(k == KH - 1))
                for k in range(KH):
                    for i in range(WAVE):
                        nc.tensor.matmul(
                            uw[i], wu_t[(k, 0)][:, i * KP:(i + 1) * KP],
                            x_t[k][:, t0:t0 + tw],
                            start=(k == 0), stop=(k == KH - 1))
                for i in range(WAVE):
                    sg = actB.tile([KP, tw], f32, tag="sg",
                                   name=f"sg{i}_{t0}")
                    nc.scalar.activation(
                        sg, gw[i], mybir.ActivationFunctionType.Silu)
                    nc.vector.tensor_mul(
                        h_t[i][:, t0:t0 + tw], sg, uw[i])

                # stage 1b: remaining token tiles of the wavefront i-tiles
                load_w_group(2)
                for i in range(WAVE):
                    load_wd(i + 3)
                    bgroup(i, ttiles[1:], 0, i)

                # stage 2: the rest, normal order
                for i in range(WAVE, KI):
                    q, r = i2q[i]
                    if r == 0 and q + 2 < NG:
                        load_w_group(q + 2)
                    wdi = i + 3
                    if wdi < KI:
                        load_wd(wdi)
                    bgroup(i, ttiles, q, r)

            # ---- phase D: yT = h @ wd, output [H, C]
            # All wd tiles are resident (streamed during phase B), so both
            # token tiles run hh-outer; each output block's copy+store is
            # staggered and the kernel tail is only the last (smallest)
            # block.
            with tc.tile_pool(name="yout", bufs=4) as yp, \
                 tc.tile_pool(name="psD", bufs=1, space="PSUM") as psD:
                for ti, (t0, tw) in enumerate(ttiles):
                    for hh in range(HH):
                        y_ps = psD.tile([KP, tw], f32, tag=f"y{hh}",
                                        name=f"y{hh}_{t0}")
                        for i in range(KI):
                            nc.tensor.matmul(
                                y_ps,
                                wd_t[i][:, hh * KP:(hh + 1) * KP],
                                h_t[i][:, t0:t0 + tw],
                                start=(i == 0), stop=(i == KI - 1))
                        yo = yp.tile([KP, tw], f32, tag="yo",
                                     name=f"yo{hh}_{t0}")
                        nc.vector.tensor_copy(yo, y_ps)
                        nc.scalar.dma_start(
                            out=yT[hh * KP:(hh + 1) * KP, t0:t0 + tw],
                            in_=yo)
    _split_multi_waits(nc)
    return nc


CMAX = 1024   # per-run token capacity (bounded by SBUF for the h tiles)


def _prepare(inputs):
    """Host-side routing + weight folding. Returns (in_maps, idx, wts, C)."""
    hs = np.asarray(inputs["hidden_states"], dtype=np.float32)
    rw = np.asarray(inputs["routing_weights"], dtype=np.float32)
    se = np.asarray(inputs["selected_experts"]).astype(np.int64)
    T = hs.shape[0]

    combine = np.zeros((T, E), dtype=np.float32)
    for k in range(se.shape[1]):
        np.add.at(combine, (np.arange(T), se[:, k]), rw[:, k])

    idx = [np.nonzero(combine[:, e])[0] for e in range(E)]
    wts = [combine[idx[e], e] for e in range(E)]
    maxn = max((len(ix) for ix in idx), default=1)
    C = min(max(512, maxn), CMAX)

    gp = np.asarray(inputs["gate_proj"], dtype=np.float32)
    up = np.asarray(inputs["up_proj"], dtype=np.float32)
    dp = np.asarray(inputs["down_proj"], dtype=np.float32)
    gA = np.asarray(inputs["gate_A"], dtype=np.float32)
    gB = np.asarray(inputs["gate_B"], dtype=np.float32)
    uA = np.asarray(inputs["up_A"], dtype=np.float32)
    uB = np.asarray(inputs["up_B"], dtype=np.float32)
    dA = np.asarray(inputs["down_A"], dtype=np.float32)
    dB = np.asarray(inputs["down_B"], dtype=np.float32)

    npdt = BF16
    wmaps = []
    for e in range(E):
        wge = (gp[e] + SCALING * (gA[e] @ gB[e])).astype(npdt)
        wue = (up[e] + SCALING * (uA[e] @ uB[e])).astype(npdt)
        wde = (dp[e] + SCALING * (dA[e] @ dB[e])).astype(npdt)
        wmaps.append({"wg": wge, "wu": wue, "wd": wde})
    return hs, wmaps, idx, wts, C, npdt


def kernel(**inputs):
    _setup_paths()
    from concourse.bass_utils import run_bass_kernel_spmd

    hs, wmaps, idx, wts, C, npdt = _prepare(inputs)

    nc = _cache.get(C)
    if nc is None:
        nc = _build(C)
        _cache[C] = nc

    T = hs.shape[0]
    out = np.zeros((T, H), dtype=np.float32)
    maxn = max((len(ix) for ix in idx), default=1)
    nruns = max(1, -(-maxn // C))
    for r in range(nruns):
        in_maps = []
        for e in range(E):
            sub = idx[e][r * C:(r + 1) * C]
            xTe = np.zeros((H, C), dtype=npdt)
            if len(sub):
                xTe[:, :len(sub)] = hs[sub].T.astype(npdt)
            in_maps.append({"xT": xTe, **wmaps[e]})
        try:
            res = run_bass_kernel_spmd(
                nc, in_maps, core_ids=list(range(NCORES)))
        except Exception:
            import time
            time.sleep(2.0)
            res = run_bass_kernel_spmd(
                nc, in_maps, core_ids=list(range(NCORES)))

        # expose for external profiling harnesses (test.py)
        kernel._last = {"nc": nc, "in_maps": in_maps, "results": res}

        for e in range(E):
            sub = idx[e][r * C:(r + 1) * C]
            if not len(sub):
                continue
            w = wts[e][r * C:(r + 1) * C]
            yTe = res.results[e]["yT"]          # [H, C] fp32
            out[sub] += w[:, None] * yTe[:, :len(sub)].T
    return out
